# revision 5
# baseline (speedup 1.0000x reference)
"""Trainium2 Bass kernel for the affine-transformer backsubstitution chain.

reference semantics (D=2048, L=8):
    Al = Au = A; bl = bu = b
    for s in 0..L-1 (history reversed):
        Al' = relu(Al) @ dAl + min(Al,0) @ dAu
        bl' = relu(Al) @ dbl + min(Al,0) @ dbu + bl
        Au' = relu(Au) @ dAu + min(Au,0) @ dAl
        bu' = relu(Au) @ dbu + min(Au,0) @ dbl + bu
    lower = relu(Al) @ lower_in + min(Al,0) @ upper_in + bl
    upper = relu(Au) @ upper_in + min(Au,0) @ lower_in + bu

Sharding: rows of Al/Au across 8 cores (256 rows each), history replicated.
Per core the state is kept TRANSPOSED ([2048 k-partitions, 256 m-free]) so the
history matrices act directly as matmul weights (out = lhsT.T @ rhs), and the
clamped copies are the state:
    mvA[k] = [ relu(AlT)[k] | min(AuT,0)[k] ]   (pairs with dAl weight tiles)
    mvB[k] = [ min(AlT,0)[k] | relu(AuT)[k] ]   (pairs with dAu weight tiles)
One [128,512] PSUM per output chunk then accumulates both chains at once:
    psum[:, :256] = sum_k dAl[k,n]·relu(AlT) + dAu[k,n]·min(AlT,0) = new AlT
    psum[:, 256:] = sum_k dAl[k,n]·min(AuT,0) + dAu[k,n]·relu(AuT) = new AuT
Compute dtype bf16 (fp32 PSUM accumulation).
"""

import numpy as np
import ml_dtypes

L = 8
D = 2048
NCORES = 8
RPC = D // NCORES  # 256 rows per core
P = 128
KC = D // P  # 16 partition chunks
W = 2 * RPC  # 512: concatenated moving width

BF16 = ml_dtypes.bfloat16

_nc_cache = {}


def _build():
    from concourse import bacc
    import concourse.tile as tile
    import concourse.mybir as mybir

    dt = mybir.dt
    nc = bacc.Bacc()

    mva0 = nc.dram_tensor("mva0", [KC, P, W], dt.bfloat16, kind="ExternalInput")
    mvb0 = nc.dram_tensor("mvb0", [KC, P, W], dt.bfloat16, kind="ExternalInput")
    hist = nc.dram_tensor("hist", [L, KC, 2, P, D], dt.bfloat16, kind="ExternalInput")
    hb = nc.dram_tensor("hb", [P, L * 2 * KC], dt.bfloat16, kind="ExternalInput")
    fin = nc.dram_tensor("fin", [P, 2 * KC], dt.bfloat16, kind="ExternalInput")
    b2 = nc.dram_tensor("b2", [1, W], dt.float32, kind="ExternalInput")
    out = nc.dram_tensor("out", [1, W], dt.float32, kind="ExternalOutput")

    with tile.TileContext(nc) as tc:
        with (
            tc.tile_pool(name="state", bufs=1) as spool,
            tc.tile_pool(name="wts", bufs=4) as wpool,
            tc.tile_pool(name="consts", bufs=1) as cpool,
            tc.tile_pool(name="bias", bufs=1) as bpool,
            tc.tile_pool(name="psum", bufs=4, space="PSUM") as ppool,
            tc.tile_pool(name="psumb", bufs=2, space="PSUM") as pbpool,
        ):
            mvA = [spool.tile([P, KC * W], dt.bfloat16, tag=f"mvA{i}", name=f"mvA{i}") for i in range(2)]
            mvB = [spool.tile([P, KC * W], dt.bfloat16, tag=f"mvB{i}", name=f"mvB{i}") for i in range(2)]
            bst = [bpool.tile([1, W], dt.float32, tag=f"b{i}", name=f"b{i}") for i in range(2)]
            hbt = cpool.tile([P, L * 2 * KC], dt.bfloat16, tag="hbt")
            fint = cpool.tile([P, 2 * KC], dt.bfloat16, tag="fint")

            for i in range(KC):
                nc.sync.dma_start(mvA[0][:, i * W : (i + 1) * W], mva0[i])
                nc.sync.dma_start(mvB[0][:, i * W : (i + 1) * W], mvb0[i])
            nc.sync.dma_start(bst[0][:], b2[:])
            nc.sync.dma_start(hbt[:], hb[:])
            nc.sync.dma_start(fint[:], fin[:])

            for s in range(L):
                cur, nxt = s % 2, (s + 1) % 2
                A, B = mvA[cur], mvB[cur]
                An, Bn = mvA[nxt], mvB[nxt]
                for j in range(KC):
                    stripe = wpool.tile([P, 2, D], dt.bfloat16, tag="stripe", name="stripe")
                    nc.sync.dma_start(stripe[:], hist[s, j].rearrange("t p f -> p t f"))
                    ps = ppool.tile([P, W], dt.float32, tag="ps", name="ps")
                    for i in range(KC):
                        nc.tensor.matmul(
                            ps[:],
                            stripe[:, 0, i * P : (i + 1) * P],
                            A[:, i * W : (i + 1) * W],
                            start=(i == 0),
                            stop=False,
                        )
                        nc.tensor.matmul(
                            ps[:],
                            stripe[:, 1, i * P : (i + 1) * P],
                            B[:, i * W : (i + 1) * W],
                            start=False,
                            stop=(i == KC - 1),
                        )
                    h = RPC
                    o = j * W
                    nc.vector.tensor_scalar_max(An[:, o : o + h], ps[:, :h], 0.0)
                    nc.vector.tensor_scalar_min(Bn[:, o : o + h], ps[:, :h], 0.0)
                    nc.vector.tensor_scalar_max(Bn[:, o + h : o + W], ps[:, h:], 0.0)
                    nc.vector.tensor_scalar_min(An[:, o + h : o + W], ps[:, h:], 0.0)
                pb = pbpool.tile([1, W], dt.float32, tag="pb", name="pb")
                for i in range(KC):
                    cl = (s * 2 + 0) * KC + i
                    cu = (s * 2 + 1) * KC + i
                    nc.tensor.matmul(
                        pb[:],
                        hbt[:, cl : cl + 1],
                        A[:, i * W : (i + 1) * W],
                        start=(i == 0),
                        stop=False,
                    )
                    nc.tensor.matmul(
                        pb[:],
                        hbt[:, cu : cu + 1],
                        B[:, i * W : (i + 1) * W],
                        start=False,
                        stop=(i == KC - 1),
                    )
                nc.vector.tensor_add(bst[nxt][:], pb[:], bst[cur][:])

            # final concretization against the input box
            Af, Bf = mvA[L % 2], mvB[L % 2]
            pf = pbpool.tile([1, W], dt.float32, tag="pb", name="pb")
            for i in range(KC):
                nc.tensor.matmul(
                    pf[:],
                    fint[:, i : i + 1],
                    Af[:, i * W : (i + 1) * W],
                    start=(i == 0),
                    stop=False,
                )
                nc.tensor.matmul(
                    pf[:],
                    fint[:, KC + i : KC + i + 1],
                    Bf[:, i * W : (i + 1) * W],
                    start=False,
                    stop=(i == KC - 1),
                )
            res = bpool.tile([1, W], dt.float32, tag="res")
            nc.vector.tensor_add(res[:], pf[:], bst[L % 2][:])
            nc.sync.dma_start(out[:], res[:])

    nc.finalize()
    return nc


def _get_nc():
    if "nc" not in _nc_cache:
        _nc_cache["nc"] = _build()
    return _nc_cache["nc"]


def _prep_inputs(A, b, hist_Al, hist_Au, hist_bl, hist_bu, lower_in, upper_in):
    A = np.asarray(A, dtype=np.float32)
    b = np.asarray(b, dtype=np.float32)
    hal = np.asarray(hist_Al, dtype=np.float32)[::-1]
    hau = np.asarray(hist_Au, dtype=np.float32)[::-1]
    hbl = np.asarray(hist_bl, dtype=np.float32)[::-1]
    hbu = np.asarray(hist_bu, dtype=np.float32)[::-1]
    lower_in = np.asarray(lower_in, dtype=np.float32)
    upper_in = np.asarray(upper_in, dtype=np.float32)

    # hist[s, j, t, p, i*P + n] = h_t[s, i*P + p, j*P + n]
    hist = np.empty([L, KC, 2, P, D], dtype=BF16)
    for t, h in enumerate((hal, hau)):
        hist[:, :, t] = (
            h.reshape(L, KC, P, KC, P).transpose(0, 3, 2, 1, 4).reshape(L, KC, P, D)
        )

    # hb[p, (s*2 + t)*KC + i] = db_t[s, i*P + p]
    hb = (
        np.stack([hbl.reshape(L, KC, P), hbu.reshape(L, KC, P)], axis=1)
        .transpose(3, 0, 1, 2)
        .reshape(P, L * 2 * KC)
        .astype(BF16)
    )

    # fin[p, t*KC + i]: t=0 lower_in, t=1 upper_in
    fin = (
        np.stack([lower_in.reshape(KC, P), upper_in.reshape(KC, P)], axis=0)
        .transpose(2, 0, 1)
        .reshape(P, 2 * KC)
        .astype(BF16)
    )

    in_maps = []
    for c in range(NCORES):
        At = np.ascontiguousarray(A[c * RPC : (c + 1) * RPC].T)  # [D, RPC]
        ap = np.maximum(At, 0.0).reshape(KC, P, RPC)
        an = np.minimum(At, 0.0).reshape(KC, P, RPC)
        mva0 = np.concatenate([ap, an], axis=2).astype(BF16)
        mvb0 = np.concatenate([an, ap], axis=2).astype(BF16)
        b_blk = b[c * RPC : (c + 1) * RPC]
        b2 = np.concatenate([b_blk, b_blk]).reshape(1, W).astype(np.float32)
        in_maps.append(
            {
                "mva0": mva0,
                "mvb0": mvb0,
                "hist": hist,
                "hb": hb,
                "fin": fin,
                "b2": b2,
            }
        )
    return in_maps


def _run(in_maps, trace=False):
    from concourse.bass_utils import run_bass_kernel_spmd

    nc = _get_nc()
    return run_bass_kernel_spmd(
        nc, in_maps, core_ids=list(range(NCORES)), trace=trace
    )


def kernel(A, b, hist_Al, hist_Au, hist_bl, hist_bu, lower_in, upper_in):
    in_maps = _prep_inputs(
        A, b, hist_Al, hist_Au, hist_bl, hist_bu, lower_in, upper_in
    )
    res = _run(in_maps, trace=False)
    lower = np.concatenate([res.results[c]["out"][0, :RPC] for c in range(NCORES)])
    upper = np.concatenate([res.results[c]["out"][0, RPC:] for c in range(NCORES)])
    return lower.astype(np.float32), upper.astype(np.float32)


# revision 7
# speedup vs baseline: 1.0080x; 1.0080x over previous
"""Trainium2 Bass kernel for the affine-transformer backsubstitution chain.

reference semantics (D=2048, L=8):
    Al = Au = A; bl = bu = b
    for s in 0..L-1 (history reversed):
        Al' = relu(Al) @ dAl + min(Al,0) @ dAu
        bl' = relu(Al) @ dbl + min(Al,0) @ dbu + bl
        Au' = relu(Au) @ dAu + min(Au,0) @ dAl
        bu' = relu(Au) @ dbu + min(Au,0) @ dbl + bu
    lower = relu(Al) @ lower_in + min(Al,0) @ upper_in + bl
    upper = relu(Au) @ upper_in + min(Au,0) @ lower_in + bu

Sharding: rows of Al/Au across 8 cores (256 rows each), history replicated.
Per core the state is kept TRANSPOSED ([2048 k-partitions, 256 m-free]) so the
history matrices act directly as matmul weights (out = lhsT.T @ rhs), and the
clamped copies are the state:
    mvA[k] = [ relu(AlT)[k] | min(AuT,0)[k] ]   (pairs with dAl weight tiles)
    mvB[k] = [ min(AlT,0)[k] | relu(AuT)[k] ]   (pairs with dAu weight tiles)
One [128,512] PSUM per output chunk then accumulates both chains at once:
    psum[:, :256] = sum_k dAl[k,n]·relu(AlT) + dAu[k,n]·min(AlT,0) = new AlT
    psum[:, 256:] = sum_k dAl[k,n]·min(AuT,0) + dAu[k,n]·relu(AuT) = new AuT
Compute dtype bf16 (fp32 PSUM accumulation).
"""

import numpy as np
import ml_dtypes

L = 8
D = 2048
NCORES = 8
RPC = D // NCORES  # 256 rows per core
P = 128
KC = D // P  # 16 partition chunks
W = 2 * RPC  # 512: concatenated moving width

BF16 = ml_dtypes.bfloat16

_nc_cache = {}


def _build():
    from concourse import bacc
    import concourse.tile as tile
    import concourse.mybir as mybir

    dt = mybir.dt
    nc = bacc.Bacc()

    mva0 = nc.dram_tensor("mva0", [KC, P, W], dt.bfloat16, kind="ExternalInput")
    mvb0 = nc.dram_tensor("mvb0", [KC, P, W], dt.bfloat16, kind="ExternalInput")
    hist = nc.dram_tensor("hist", [L, KC, 2, P, D], dt.bfloat16, kind="ExternalInput")
    hb = nc.dram_tensor("hb", [P, L * 2 * KC], dt.bfloat16, kind="ExternalInput")
    fin = nc.dram_tensor("fin", [P, 2 * KC], dt.bfloat16, kind="ExternalInput")
    b2 = nc.dram_tensor("b2", [1, W], dt.float32, kind="ExternalInput")
    out = nc.dram_tensor("out", [1, W], dt.float32, kind="ExternalOutput")

    with tile.TileContext(nc) as tc:
        with (
            tc.tile_pool(name="state", bufs=1) as spool,
            tc.tile_pool(name="wts", bufs=4) as wpool,
            tc.tile_pool(name="consts", bufs=1) as cpool,
            tc.tile_pool(name="bias", bufs=1) as bpool,
            tc.tile_pool(name="psum", bufs=4, space="PSUM") as ppool,
            tc.tile_pool(name="psumb", bufs=2, space="PSUM") as pbpool,
        ):
            mvA = [spool.tile([P, KC * W], dt.bfloat16, tag=f"mvA{i}", name=f"mvA{i}") for i in range(2)]
            mvB = [spool.tile([P, KC * W], dt.bfloat16, tag=f"mvB{i}", name=f"mvB{i}") for i in range(2)]
            bst = [bpool.tile([1, W], dt.float32, tag=f"b{i}", name=f"b{i}") for i in range(2)]
            hbt = cpool.tile([P, L * 2 * KC], dt.bfloat16, tag="hbt")
            fint = cpool.tile([P, 2 * KC], dt.bfloat16, tag="fint")

            # Issue the first weight stripe before the state init so the PE can
            # start as soon as state chunk 0 lands; then feed state chunks in
            # consumption order. hbt/fint/b2 are not needed until the end of
            # step 0, so they go last.
            stripes = {}
            stripes[(0, 0)] = wpool.tile([P, 2, D], dt.bfloat16, tag="stripe", name="stripe")
            nc.sync.dma_start(stripes[(0, 0)][:], hist[0, 0].rearrange("t p f -> p t f"))
            for i in range(KC):
                nc.sync.dma_start(mvA[0][:, i * W : (i + 1) * W], mva0[i])
                nc.sync.dma_start(mvB[0][:, i * W : (i + 1) * W], mvb0[i])
            nc.sync.dma_start(bst[0][:], b2[:])
            nc.sync.dma_start(hbt[:], hb[:])
            nc.sync.dma_start(fint[:], fin[:])

            for s in range(L):
                cur, nxt = s % 2, (s + 1) % 2
                A, B = mvA[cur], mvB[cur]
                An, Bn = mvA[nxt], mvB[nxt]
                for j in range(KC):
                    if (s, j) in stripes:
                        stripe = stripes.pop((s, j))
                    else:
                        stripe = wpool.tile([P, 2, D], dt.bfloat16, tag="stripe", name="stripe")
                        nc.sync.dma_start(stripe[:], hist[s, j].rearrange("t p f -> p t f"))
                    ps = ppool.tile([P, W], dt.float32, tag="ps", name="ps")
                    for i in range(KC):
                        nc.tensor.matmul(
                            ps[:],
                            stripe[:, 0, i * P : (i + 1) * P],
                            A[:, i * W : (i + 1) * W],
                            start=(i == 0),
                            stop=False,
                        )
                        nc.tensor.matmul(
                            ps[:],
                            stripe[:, 1, i * P : (i + 1) * P],
                            B[:, i * W : (i + 1) * W],
                            start=False,
                            stop=(i == KC - 1),
                        )
                    h = RPC
                    o = j * W
                    nc.vector.tensor_scalar_max(An[:, o : o + h], ps[:, :h], 0.0)
                    nc.vector.tensor_scalar_min(Bn[:, o : o + h], ps[:, :h], 0.0)
                    nc.vector.tensor_scalar_max(Bn[:, o + h : o + W], ps[:, h:], 0.0)
                    nc.vector.tensor_scalar_min(An[:, o + h : o + W], ps[:, h:], 0.0)
                pb = pbpool.tile([1, W], dt.float32, tag="pb", name="pb")
                for i in range(KC):
                    cl = (s * 2 + 0) * KC + i
                    cu = (s * 2 + 1) * KC + i
                    nc.tensor.matmul(
                        pb[:],
                        hbt[:, cl : cl + 1],
                        A[:, i * W : (i + 1) * W],
                        start=(i == 0),
                        stop=False,
                    )
                    nc.tensor.matmul(
                        pb[:],
                        hbt[:, cu : cu + 1],
                        B[:, i * W : (i + 1) * W],
                        start=False,
                        stop=(i == KC - 1),
                    )
                nc.vector.tensor_add(bst[nxt][:], pb[:], bst[cur][:])

            # final concretization against the input box
            Af, Bf = mvA[L % 2], mvB[L % 2]
            pf = pbpool.tile([1, W], dt.float32, tag="pb", name="pb")
            for i in range(KC):
                nc.tensor.matmul(
                    pf[:],
                    fint[:, i : i + 1],
                    Af[:, i * W : (i + 1) * W],
                    start=(i == 0),
                    stop=False,
                )
                nc.tensor.matmul(
                    pf[:],
                    fint[:, KC + i : KC + i + 1],
                    Bf[:, i * W : (i + 1) * W],
                    start=False,
                    stop=(i == KC - 1),
                )
            res = bpool.tile([1, W], dt.float32, tag="res")
            nc.vector.tensor_add(res[:], pf[:], bst[L % 2][:])
            nc.sync.dma_start(out[:], res[:])

    nc.finalize()
    return nc


def _get_nc():
    if "nc" not in _nc_cache:
        _nc_cache["nc"] = _build()
    return _nc_cache["nc"]


def _prep_inputs(A, b, hist_Al, hist_Au, hist_bl, hist_bu, lower_in, upper_in):
    A = np.asarray(A, dtype=np.float32)
    b = np.asarray(b, dtype=np.float32)
    hal = np.asarray(hist_Al, dtype=np.float32)[::-1]
    hau = np.asarray(hist_Au, dtype=np.float32)[::-1]
    hbl = np.asarray(hist_bl, dtype=np.float32)[::-1]
    hbu = np.asarray(hist_bu, dtype=np.float32)[::-1]
    lower_in = np.asarray(lower_in, dtype=np.float32)
    upper_in = np.asarray(upper_in, dtype=np.float32)

    # hist[s, j, t, p, i*P + n] = h_t[s, i*P + p, j*P + n]
    hist = np.empty([L, KC, 2, P, D], dtype=BF16)
    for t, h in enumerate((hal, hau)):
        hist[:, :, t] = (
            h.reshape(L, KC, P, KC, P).transpose(0, 3, 2, 1, 4).reshape(L, KC, P, D)
        )

    # hb[p, (s*2 + t)*KC + i] = db_t[s, i*P + p]
    hb = (
        np.stack([hbl.reshape(L, KC, P), hbu.reshape(L, KC, P)], axis=1)
        .transpose(3, 0, 1, 2)
        .reshape(P, L * 2 * KC)
        .astype(BF16)
    )

    # fin[p, t*KC + i]: t=0 lower_in, t=1 upper_in
    fin = (
        np.stack([lower_in.reshape(KC, P), upper_in.reshape(KC, P)], axis=0)
        .transpose(2, 0, 1)
        .reshape(P, 2 * KC)
        .astype(BF16)
    )

    in_maps = []
    for c in range(NCORES):
        At = np.ascontiguousarray(A[c * RPC : (c + 1) * RPC].T)  # [D, RPC]
        ap = np.maximum(At, 0.0).reshape(KC, P, RPC)
        an = np.minimum(At, 0.0).reshape(KC, P, RPC)
        mva0 = np.concatenate([ap, an], axis=2).astype(BF16)
        mvb0 = np.concatenate([an, ap], axis=2).astype(BF16)
        b_blk = b[c * RPC : (c + 1) * RPC]
        b2 = np.concatenate([b_blk, b_blk]).reshape(1, W).astype(np.float32)
        in_maps.append(
            {
                "mva0": mva0,
                "mvb0": mvb0,
                "hist": hist,
                "hb": hb,
                "fin": fin,
                "b2": b2,
            }
        )
    return in_maps


def _run(in_maps, trace=False):
    from concourse.bass_utils import run_bass_kernel_spmd

    nc = _get_nc()
    return run_bass_kernel_spmd(
        nc, in_maps, core_ids=list(range(NCORES)), trace=trace
    )


def kernel(A, b, hist_Al, hist_Au, hist_bl, hist_bu, lower_in, upper_in):
    in_maps = _prep_inputs(
        A, b, hist_Al, hist_Au, hist_bl, hist_bu, lower_in, upper_in
    )
    res = _run(in_maps, trace=False)
    lower = np.concatenate([res.results[c]["out"][0, :RPC] for c in range(NCORES)])
    upper = np.concatenate([res.results[c]["out"][0, RPC:] for c in range(NCORES)])
    return lower.astype(np.float32), upper.astype(np.float32)


# revision 13
# speedup vs baseline: 1.0152x; 1.0071x over previous
"""Trainium2 Bass kernel for the affine-transformer backsubstitution chain.

reference semantics (D=2048, L=8):
    Al = Au = A; bl = bu = b
    for s in 0..L-1 (history reversed):
        Al' = relu(Al) @ dAl + min(Al,0) @ dAu
        bl' = relu(Al) @ dbl + min(Al,0) @ dbu + bl
        Au' = relu(Au) @ dAu + min(Au,0) @ dAl
        bu' = relu(Au) @ dbu + min(Au,0) @ dbl + bu
    lower = relu(Al) @ lower_in + min(Al,0) @ upper_in + bl
    upper = relu(Au) @ upper_in + min(Au,0) @ lower_in + bu

Sharding: rows of Al/Au across 8 cores (256 rows each), history replicated.
Per core the state is kept TRANSPOSED ([2048 k-partitions, 256 m-free]) so the
history matrices act directly as matmul weights (out = lhsT.T @ rhs), and the
clamped copies are the state:
    mvA[k] = [ relu(AlT)[k] | min(AuT,0)[k] ]   (pairs with dAl weight tiles)
    mvB[k] = [ min(AlT,0)[k] | relu(AuT)[k] ]   (pairs with dAu weight tiles)
One [128,512] PSUM per output chunk then accumulates both chains at once:
    psum[:, :256] = sum_k dAl[k,n]·relu(AlT) + dAu[k,n]·min(AlT,0) = new AlT
    psum[:, 256:] = sum_k dAl[k,n]·min(AuT,0) + dAu[k,n]·relu(AuT) = new AuT
Compute dtype bf16 (fp32 PSUM accumulation).
"""

import numpy as np
import ml_dtypes

L = 8
D = 2048
NCORES = 8
RPC = D // NCORES  # 256 rows per core
P = 128
KC = D // P  # 16 partition chunks
W = 2 * RPC  # 512: concatenated moving width

BF16 = ml_dtypes.bfloat16

_nc_cache = {}


def _build():
    from concourse import bacc
    import concourse.tile as tile
    import concourse.mybir as mybir

    dt = mybir.dt
    nc = bacc.Bacc()

    mva0 = nc.dram_tensor("mva0", [KC, P, W], dt.bfloat16, kind="ExternalInput")
    hist = nc.dram_tensor("hist", [L, KC, 2, P, D], dt.bfloat16, kind="ExternalInput")
    hb = nc.dram_tensor("hb", [P, L * 2 * KC], dt.bfloat16, kind="ExternalInput")
    fin = nc.dram_tensor("fin", [P, 2 * KC], dt.bfloat16, kind="ExternalInput")
    b2 = nc.dram_tensor("b2", [1, W], dt.float32, kind="ExternalInput")
    out = nc.dram_tensor("out", [1, W], dt.float32, kind="ExternalOutput")

    with tile.TileContext(nc) as tc:
        with (
            tc.tile_pool(name="state", bufs=1) as spool,
            tc.tile_pool(name="wts", bufs=6) as wpool,
            tc.tile_pool(name="consts", bufs=1) as cpool,
            tc.tile_pool(name="bias", bufs=1) as bpool,
            tc.tile_pool(name="psum", bufs=5, space="PSUM") as ppool,
            tc.tile_pool(name="psumb", bufs=2, space="PSUM") as pbpool,
            tc.tile_pool(name="psumw", bufs=1, space="PSUM") as pwpool,
        ):
            mvA = [spool.tile([P, KC * W], dt.bfloat16, tag=f"mvA{i}", name=f"mvA{i}") for i in range(2)]
            mvB = [spool.tile([P, KC * W], dt.bfloat16, tag=f"mvB{i}", name=f"mvB{i}") for i in range(2)]
            bst = [bpool.tile([1, W], dt.float32, tag=f"b{i}", name=f"b{i}") for i in range(2)]
            hbt = cpool.tile([P, L * 2 * KC], dt.bfloat16, tag="hbt")
            fint = cpool.tile([P, 2 * KC], dt.bfloat16, tag="fint")

            # PE warmup: ~64 cheap matmuls on a zeroed tile run during the
            # initial DMA window so HAM un-throttles before the real stream.
            warm = cpool.tile([P, W], dt.bfloat16, tag="warm")
            nc.vector.memset(warm[:], 0.0)
            pw = pwpool.tile([P, P], dt.float32, tag="pw", name="pw")
            for i in range(64):
                nc.tensor.matmul(pw[:], warm[:, :P], warm[:, :P], start=True, stop=True)

            # Issue the first weight stripe first on the Sync queue; state init
            # goes on the Scalar HWDGE queue so stripe prefetches aren't stuck
            # behind it. mvB is mvA with the halves swapped — derive it on the
            # (idle) DVE instead of DMAing another 2MB.
            stripes = {}
            stripes[(0, 0)] = wpool.tile([P, 2, D], dt.bfloat16, tag="stripe", name="stripe")
            nc.sync.dma_start(stripes[(0, 0)][:], hist[0, 0].rearrange("t p f -> p t f"))
            for i in range(KC):
                o = i * W
                nc.scalar.dma_start(mvA[0][:, o : o + W], mva0[i])
                nc.vector.tensor_copy(mvB[0][:, o : o + RPC], mvA[0][:, o + RPC : o + W])
                nc.vector.tensor_copy(mvB[0][:, o + RPC : o + W], mvA[0][:, o : o + RPC])
            nc.scalar.dma_start(bst[0][:], b2[:])
            nc.scalar.dma_start(hbt[:], hb[:])
            nc.scalar.dma_start(fint[:], fin[:])

            for s in range(L):
                cur, nxt = s % 2, (s + 1) % 2
                A, B = mvA[cur], mvB[cur]
                An, Bn = mvA[nxt], mvB[nxt]
                for j in range(KC):
                    if (s, j) in stripes:
                        stripe = stripes.pop((s, j))
                    else:
                        stripe = wpool.tile([P, 2, D], dt.bfloat16, tag="stripe", name="stripe")
                        nc.sync.dma_start(stripe[:], hist[s, j].rearrange("t p f -> p t f"))
                    ps = ppool.tile([P, W], dt.float32, tag="ps", name="ps")
                    for i in range(KC):
                        nc.tensor.matmul(
                            ps[:],
                            stripe[:, 0, i * P : (i + 1) * P],
                            A[:, i * W : (i + 1) * W],
                            start=(i == 0),
                            stop=False,
                        )
                        nc.tensor.matmul(
                            ps[:],
                            stripe[:, 1, i * P : (i + 1) * P],
                            B[:, i * W : (i + 1) * W],
                            start=False,
                            stop=(i == KC - 1),
                        )
                    h = RPC
                    o = j * W
                    nc.vector.tensor_scalar_max(An[:, o : o + h], ps[:, :h], 0.0)
                    nc.vector.tensor_scalar_min(Bn[:, o : o + h], ps[:, :h], 0.0)
                    nc.vector.tensor_scalar_max(Bn[:, o + h : o + W], ps[:, h:], 0.0)
                    nc.vector.tensor_scalar_min(An[:, o + h : o + W], ps[:, h:], 0.0)
                pb = pbpool.tile([1, W], dt.float32, tag="pb", name="pb")
                for i in range(KC):
                    cl = (s * 2 + 0) * KC + i
                    cu = (s * 2 + 1) * KC + i
                    nc.tensor.matmul(
                        pb[:],
                        hbt[:, cl : cl + 1],
                        A[:, i * W : (i + 1) * W],
                        start=(i == 0),
                        stop=False,
                    )
                    nc.tensor.matmul(
                        pb[:],
                        hbt[:, cu : cu + 1],
                        B[:, i * W : (i + 1) * W],
                        start=False,
                        stop=(i == KC - 1),
                    )
                nc.vector.tensor_add(bst[nxt][:], pb[:], bst[cur][:])

            # final concretization against the input box
            Af, Bf = mvA[L % 2], mvB[L % 2]
            pf = pbpool.tile([1, W], dt.float32, tag="pb", name="pb")
            for i in range(KC):
                nc.tensor.matmul(
                    pf[:],
                    fint[:, i : i + 1],
                    Af[:, i * W : (i + 1) * W],
                    start=(i == 0),
                    stop=False,
                )
                nc.tensor.matmul(
                    pf[:],
                    fint[:, KC + i : KC + i + 1],
                    Bf[:, i * W : (i + 1) * W],
                    start=False,
                    stop=(i == KC - 1),
                )
            res = bpool.tile([1, W], dt.float32, tag="res")
            nc.vector.tensor_add(res[:], pf[:], bst[L % 2][:])
            nc.sync.dma_start(out[:], res[:])

    nc.finalize()
    return nc


def _get_nc():
    if "nc" not in _nc_cache:
        _nc_cache["nc"] = _build()
    return _nc_cache["nc"]


def _prep_inputs(A, b, hist_Al, hist_Au, hist_bl, hist_bu, lower_in, upper_in):
    A = np.asarray(A, dtype=np.float32)
    b = np.asarray(b, dtype=np.float32)
    hal = np.asarray(hist_Al, dtype=np.float32)[::-1]
    hau = np.asarray(hist_Au, dtype=np.float32)[::-1]
    hbl = np.asarray(hist_bl, dtype=np.float32)[::-1]
    hbu = np.asarray(hist_bu, dtype=np.float32)[::-1]
    lower_in = np.asarray(lower_in, dtype=np.float32)
    upper_in = np.asarray(upper_in, dtype=np.float32)

    # hist[s, j, t, p, i*P + n] = h_t[s, i*P + p, j*P + n]
    hist = np.empty([L, KC, 2, P, D], dtype=BF16)
    for t, h in enumerate((hal, hau)):
        hist[:, :, t] = (
            h.reshape(L, KC, P, KC, P).transpose(0, 3, 2, 1, 4).reshape(L, KC, P, D)
        )

    # hb[p, (s*2 + t)*KC + i] = db_t[s, i*P + p]
    hb = (
        np.stack([hbl.reshape(L, KC, P), hbu.reshape(L, KC, P)], axis=1)
        .transpose(3, 0, 1, 2)
        .reshape(P, L * 2 * KC)
        .astype(BF16)
    )

    # fin[p, t*KC + i]: t=0 lower_in, t=1 upper_in
    fin = (
        np.stack([lower_in.reshape(KC, P), upper_in.reshape(KC, P)], axis=0)
        .transpose(2, 0, 1)
        .reshape(P, 2 * KC)
        .astype(BF16)
    )

    in_maps = []
    for c in range(NCORES):
        At = np.ascontiguousarray(A[c * RPC : (c + 1) * RPC].T)  # [D, RPC]
        ap = np.maximum(At, 0.0).reshape(KC, P, RPC)
        an = np.minimum(At, 0.0).reshape(KC, P, RPC)
        mva0 = np.concatenate([ap, an], axis=2).astype(BF16)
        b_blk = b[c * RPC : (c + 1) * RPC]
        b2 = np.concatenate([b_blk, b_blk]).reshape(1, W).astype(np.float32)
        in_maps.append(
            {
                "mva0": mva0,
                "hist": hist,
                "hb": hb,
                "fin": fin,
                "b2": b2,
            }
        )
    return in_maps


def _run(in_maps, trace=False):
    from concourse.bass_utils import run_bass_kernel_spmd

    nc = _get_nc()
    return run_bass_kernel_spmd(
        nc, in_maps, core_ids=list(range(NCORES)), trace=trace
    )


def kernel(A, b, hist_Al, hist_Au, hist_bl, hist_bu, lower_in, upper_in):
    in_maps = _prep_inputs(
        A, b, hist_Al, hist_Au, hist_bl, hist_bu, lower_in, upper_in
    )
    res = _run(in_maps, trace=False)
    lower = np.concatenate([res.results[c]["out"][0, :RPC] for c in range(NCORES)])
    upper = np.concatenate([res.results[c]["out"][0, RPC:] for c in range(NCORES)])
    return lower.astype(np.float32), upper.astype(np.float32)
